# revision 45
# baseline (speedup 1.0000x reference)
"""BailingMoE linear attention (lightning attention) on 8 trn2 NeuronCores.

Tensor-parallel over heads (2 heads/core), fp16 matmul operands, fp32
PSUM. Software-pipelined: body b runs the qkv/g projections of group b
on the PE while the RMSNorm/RoPE/attention chains of group b-1 run on
Act/DVE/GpSimd, so the PE never drains on the norm chains. All scalar
activations live in one HW table set (ln/exp/square/identity/copy):
the sigmoid gate is 1/(1+exp(-g)) with a DVE divide, so the activation
table is loaded exactly once. The RoPE half-swap runs as SBUF->SBUF
DMAs instead of PE permutation matmuls. Weight/input DMAs are split so
the first projection starts as soon as the first contraction tile
lands. The head->sequence AllToAll splits into 8 half-chunk pieces
(128 rows per core each) so collectives fire every other body and the
final piece's collective+dense tail is half as long.
"""
import math

import numpy as np

S = 8192
HID = 2048
H = 16
D = 128
BLK = 256
GROUPS = 8
EPS = 1e-5
ROPE_THETA = 600000.0
SCALE = D ** -0.5
N_CORES = 8
HPC = H // N_CORES          # heads per core = 2
CPC = HPC * D               # channels per core = 256
KT = 16                     # contraction tiles (2048 hid; bias via ACT)
SEQ_G = 512                 # seq per projection group
NG = S // SEQ_G             # 16 groups
SB = S // N_CORES           # seq rows per core in the output = 1024
CPG = SEQ_G // BLK          # chunks per group = 2
NCH = S // BLK              # 32 chunks
NP = 8                      # a2a/dense pieces (8 x 128-row units/core)

_cache = {}


def _build_slopes():
    start = 2.0 ** (-(2.0 ** (-(math.log2(H) - 3.0))))
    slopes = np.array([start * start ** i for i in range(H)], dtype=np.float32)
    return slopes * np.float32(1.0 - 0.0 / (20 - 1) + 1e-5)


def _build_program():
    import concourse.bacc as bacc
    import concourse.tile as tile
    import concourse.mybir as mybir
    from contextlib import ExitStack

    dt = mybir.dt
    AF = mybir.ActivationFunctionType
    OP = mybir.AluOpType

    nc = bacc.Bacc("TRN2", target_bir_lowering=False, debug=False,
                   num_devices=N_CORES)

    def din(name, shape, dtype=dt.float32):
        return nc.dram_tensor(name, shape, dtype, kind="ExternalInput").ap()

    hsT = din("hsT", [KT * 128, S], dt.float16)
    wB = din("wB", [KT * 128, 768], dt.float16)   # cols: q(256) k(256) g(256)
    wv = din("wv", [KT * 128, 256], dt.float16)
    dwT = din("dwT", [HID, HID], dt.float16)
    cosf = din("cosf", [128, S], dt.float16)       # [cos; cos]
    sinf = din("sinf", [128, S], dt.float16)       # [-sin; sin]
    qdec_d = din("qdec", [128, HPC, BLK], dt.float16)
    kdec_d = din("kdec", [128, HPC, BLK], dt.float16)
    diag_d = din("diagT", [128, HPC, 2, BLK], dt.float16)
    qnw_d = din("qnw", [128, 1])                   # q_norm_w * SCALE
    knw_d = din("knw", [128, 1])
    qkb_d = din("qkb", [128, 4])                   # qb0 kb0 qb1 kb1
    vb_d = din("vbias", [128, 256], dt.float16)    # v bias bcast over seq
    gnw_d = din("gnw", [128, HPC])
    blk_d = din("blkdec", [128, HPC])
    ones128_d = din("ones128", [128, 1], dt.float16)
    idm_d = din("idm", [128, 128], dt.float16)
    zkv_d = din("zkv", [128, 128], dt.float32)

    out_d = nc.dram_tensor("out", [SB, HID], dt.float16,
                           kind="ExternalOutput").ap()

    with tile.TileContext(nc) as tc:
        ctx = ExitStack()
        consts = ctx.enter_context(tc.tile_pool(name="consts", bufs=1))
        wpool = ctx.enter_context(tc.tile_pool(name="wpool", bufs=1))
        dramp = ctx.enter_context(tc.tile_pool(name="dramp", bufs=1,
                                               space="DRAM"))
        y_send = [dramp.tile([N_CORES, CPC, 128], dt.float16,
                             name=f"y_send{i}", tag=f"y_send{i}")
                  for i in range(NP)]
        y_recv = [dramp.tile([N_CORES, CPC, 128], dt.float16,
                             name=f"y_recv{i}", tag=f"y_recv{i}")
                  for i in range(NP)]

        def cload(name, ap_src, shape, dtype=dt.float32):
            t = consts.tile(shape, dtype, name=name, tag=name)
            nc.sync.dma_start(out=t[:], in_=ap_src)
            return t

        # weights first, split by output-column block so chain ci starts
        # as soon as its 0.5MB slice lands; everything not needed by the
        # first projection group is issued after dma_group(0)
        wB_sb = wpool.tile([128, KT, 768], dt.float16, name="wB_sb")
        wB_r = wB.rearrange("(t p) c -> p t c", p=128)
        nc.sync.dma_start(out=wB_sb[:, :, 0:128], in_=wB_r[:, :, 0:128])
        wv_sb = wpool.tile([128, KT, 256], dt.float16, name="wv_sb")
        wv_r = wv.rearrange("(t p) c -> p t c", p=128)
        dwts = [wpool.tile([128, KT, 1024], dt.float16, name=f"dwt{hh}",
                           tag=f"dw{hh}") for hh in range(2)]
        dummy_s = dramp.tile([N_CORES, 4], dt.float16, name="dummy_s",
                             tag="dummy_s")
        dummy_r = dramp.tile([N_CORES, 4], dt.float16, name="dummy_r",
                             tag="dummy_r")


        hkp = ctx.enter_context(tc.tile_pool(name="hkp", bufs=2))
        tabp = ctx.enter_context(tc.tile_pool(name="tabp", bufs=1))
        evp = ctx.enter_context(tc.tile_pool(name="evp", bufs=2))
        xrp = ctx.enter_context(tc.tile_pool(name="xrp", bufs=1))
        natp = ctx.enter_context(tc.tile_pool(name="natp", bufs=1))
        attp = ctx.enter_context(tc.tile_pool(name="attp", bufs=2))
        kvpl = ctx.enter_context(tc.tile_pool(name="kvpl", bufs=1))
        yp = ctx.enter_context(tc.tile_pool(name="yp", bufs=2))
        dyp = ctx.enter_context(tc.tile_pool(name="dyp", bufs=1))
        dop = ctx.enter_context(tc.tile_pool(name="dop", bufs=1))
        psp = ctx.enter_context(tc.tile_pool(name="psp", bufs=2, space="PSUM"))
        pse = ctx.enter_context(tc.tile_pool(name="pse", bufs=2, space="PSUM"))
        psa = ctx.enter_context(tc.tile_pool(name="psa", bufs=2, space="PSUM"))
        dps = ctx.enter_context(tc.tile_pool(name="dps", bufs=2, space="PSUM"))

        # persistent kv state: fp32 master + fp16 matmul copy, ping-pong
        kv_sb = [[kvpl.tile([128, 128], dt.float32, name=f"kv{h}_{i}",
                            tag=f"kv{h}_{i}") for i in range(2)]
                 for h in range(HPC)]
        kv16 = [[kvpl.tile([128, 128], dt.float16, name=f"kv16_{h}_{i}",
                           tag=f"kv16_{h}_{i}") for i in range(2)]
                for h in range(HPC)]

        yt_cache = {}

        def dense_piece(p, hqs=(0, 1, 2, 3)):
            if p in yt_cache:
                yt = yt_cache[p]
            else:
                yt = dyp.tile([128, N_CORES, 2, 128], dt.float16,
                              name=f"yt{p}", tag="yt")
                nc.sync.dma_start(
                    out=yt[:],
                    in_=y_recv[p].rearrange("s (h p) q -> p s h q", p=128))
                yt_cache[p] = yt
            for hq in hqs:
                acc = dps.tile([128, 512], dt.float32,
                               name=f"dacc{p}_{hq}", tag="dps")
                for ct in range(2 * N_CORES):
                    nc.tensor.matmul(
                        acc[:],
                        yt[:, ct // 2, ct % 2, :],
                        dwts[hq // 2][:, ct,
                                      (hq % 2) * 512:(hq % 2 + 1) * 512],
                        start=(ct == 0), stop=(ct == 2 * N_CORES - 1))
                ot = dop.tile([128, 512], dt.float16,
                              name=f"ot{p}_{hq}", tag="ot", bufs=2)
                nc.scalar.activation(ot[:], acc[:], AF.Copy)
                srow = p * 128
                nc.sync.dma_start(
                    out=out_d[srow:srow + 128,
                              hq * 512:(hq + 1) * 512],
                    in_=ot[:])

        def dma_group(g):
            hk = hkp.tile([128, KT, SEQ_G], dt.float16,
                          name=f"hk{g}", tag="hk")
            hs_r = hsT.rearrange("(t p) s -> p t s", p=128)
            for i in range(4):
                nc.sync.dma_start(
                    out=hk[:, i * 4:(i + 1) * 4, :],
                    in_=hs_r[:, i * 4:(i + 1) * 4,
                             g * SEQ_G:(g + 1) * SEQ_G])
            cos_g = tabp.tile([128, SEQ_G], dt.float16, name=f"cos{g}",
                              tag="cos")
            nc.sync.dma_start(out=cos_g[:],
                              in_=cosf[:, g * SEQ_G:(g + 1) * SEQ_G])
            sin_g = tabp.tile([128, SEQ_G], dt.float16, name=f"sin{g}",
                              tag="sin")
            nc.sync.dma_start(out=sin_g[:],
                              in_=sinf[:, g * SEQ_G:(g + 1) * SEQ_G])
            return {"hk": hk, "cos": cos_g, "sin": sin_g}

        def proj_chain(st, ci, acc_slice, hk):
            # one 16-matmul accumulation writing acc_slice
            if ci is not None:   # q/k/g chain: weights stationary
                for t in range(KT):
                    nc.tensor.matmul(
                        acc_slice, wB_sb[:, t, ci * 128:(ci + 1) * 128],
                        hk[:, t, :], start=(t == 0), stop=(t == KT - 1))
            else:                # v chain: hk seq-slice stationary
                for t in range(KT):
                    nc.tensor.matmul(
                        acc_slice, hk[:, t, st * 128:(st + 1) * 128],
                        wv_sb[:, t, :], start=(t == 0), stop=(t == KT - 1))

        epsb4 = consts.tile([4, 1], dt.float32, name="epsb4", tag="epsb4")
        nc.vector.memset(epsb4[:], EPS)

        def emit_norm_a(p, cj0):
            # two sum-of-squares matmuls + fast DVE reciprocal (fp32)
            for cj in (cj0, cj0 + 1):
                ssq = psa.tile([1, SEQ_G], dt.float32,
                               name=f"ssq{p['g']}_{cj}", tag="psa")
                lnt = evp.tile([1, SEQ_G], dt.float32,
                               name=f"ln{p['g']}_{cj}", tag="ln", bufs=3)
                nc.tensor.matmul(ssq[:], ones128[:], p["sq"][cj][:],
                                 start=True, stop=True)
                nc.vector.reciprocal_approx_fast(out=lnt[:], in_=ssq[:])
                p.setdefault("ln", []).append(lnt)

        def emit_norm_c(p):
            rstds = []
            for cj in range(4):
                rstd = evp.tile([1, SEQ_G], dt.float16,
                                name=f"rstd{p['g']}_{cj}", tag="rstd", bufs=2)
                # rstd = sqrt(D/ssq); eps negligible vs ssq/D ~ O(1)
                nc.scalar.activation(rstd[:], p["ln"][cj][:], AF.Sqrt,
                                     scale=float(D))
                rstds.append(rstd)
            p["sg"] = []
            for i in range(2):
                sg = xrp.tile([128, SEQ_G], dt.float16,
                              name=f"sg_{p['g']}_{i}", tag=f"th1{i}",
                              bufs=1)
                nc.vector.tensor_scalar(sg[:], p["th"][i][:], 0.5, 0.5,
                                        OP.mult, OP.add)
                p["sg"].append(sg)
            p["xn"] = []
            p["m1"] = []
            for cj in range(4):
                rbc = evp.tile([128, SEQ_G], dt.float16,
                               name=f"rbc{p['g']}_{cj}", tag="rbc", bufs=2)
                nc.gpsimd.partition_broadcast(rbc[:], rstds[cj][:])
                xn = evp.tile([128, SEQ_G], dt.float16,
                              name=f"xn{p['g']}_{cj}", tag="xn", bufs=4)
                nc.vector.scalar_tensor_tensor(
                    out=xn[:], in0=p["xb"][cj][:],
                    scalar=qnw[:] if cj < 2 else knw[:],
                    in1=rbc[:], op0=OP.mult, op1=OP.mult)
                p["xn"].append(xn)
                m1 = evp.tile([128, SEQ_G], dt.float16,
                              name=f"m1{p['g']}_{cj}", tag="m1", bufs=3)
                nc.vector.tensor_tensor(out=m1[:], in0=xn[:],
                                        in1=p["cos"][:], op=OP.mult)
                p["m1"].append(m1)

        def emit_swap(p, b):
            # RoPE half-swap via SBUF->SBUF DMA (sign baked into sinf)
            p["swp"] = []
            for cj in range(4):
                swp = xrp.tile([128, SEQ_G], dt.float16,
                               name=f"swp{b}_{cj}", tag=f"swp{cj % 2}",
                               bufs=1)
                nc.sync.dma_start(out=swp[0:64, :], in_=p["xn"][cj][64:128, :])
                nc.sync.dma_start(out=swp[64:128, :], in_=p["xn"][cj][0:64, :])
                p["swp"].append(swp)

        def emit_rope_tail(p):
            p["xr"] = []   # cj 0,1 = q heads; 2,3 = k heads
            for cj in range(4):
                m2 = evp.tile([128, SEQ_G], dt.float16,
                              name=f"m2{p['g']}_{cj}", tag="m2", bufs=1)
                nc.vector.tensor_tensor(out=m2[:], in0=p["swp"][cj][:],
                                        in1=p["sin"][:], op=OP.mult)
                xr = xrp.tile([128, SEQ_G], dt.float16,
                              name=f"xr{p['g']}_{cj}", tag=f"xr{cj}", bufs=1)
                nc.vector.tensor_tensor(out=xr[:], in0=p["m1"][cj][:],
                                        in1=m2[:], op=OP.add)
                p["xr"].append(xr)
            p["ktil"] = []
            for h in range(HPC):
                ktil = xrp.tile([128, SEQ_G], dt.float16,
                                name=f"ktil{p['g']}_{h}", tag=f"ktil{h}",
                                bufs=1)
                for cc in range(CPG):
                    nc.vector.tensor_tensor(
                        out=ktil[:, cc * BLK:(cc + 1) * BLK],
                        in0=p["xr"][2 + h][:, cc * BLK:(cc + 1) * BLK],
                        in1=kdec[:, h, :], op=OP.mult)
                p["ktil"].append(ktil)

        def emit_knat_attn(p):
            g = p["g"]
            knat = [[None] * 2 for _ in range(HPC)]
            for h in range(HPC):
                for cc in range(CPG):
                    for j in range(2):
                        tp = pse.tile([128, 128], dt.float16,
                                      name=f"tp{g}_{h}_{cc}_{j}", tag="swp",
                                      bufs=2)
                        nc.tensor.transpose(
                            tp[:],
                            p["ktil"][h][:, cc * BLK + j * 128:
                                         cc * BLK + (j + 1) * 128],
                            idm[:])
                        kn = natp.tile([128, 128], dt.float16,
                                       name=f"kn{g}_{h}_{cc}_{j}", tag="kn",
                                       bufs=6)
                        nc.vector.tensor_copy(kn[:], tp[:])
                        if knat[h][cc] is None:
                            knat[h][cc] = []
                        knat[h][cc].append(kn)
            gssqs = [pse.tile([1, BLK], dt.float32, name=f"gssq{g}_{cc}",
                              tag="swp") for cc in range(CPG)]
            obs = [[None] * HPC for _ in range(CPG)]
            for cc in range(CPG):
                ch = g * CPG + cc
                # phase 1: kq + masks + decayed q, both heads
                kqd = [[None, None] for _ in range(HPC)]
                qts = []
                for h in range(HPC):
                    qr = p["xr"][h][:, cc * BLK:(cc + 1) * BLK]
                    for j in range(2):
                        kq = psa.tile([128, BLK], dt.float32,
                                      name=f"kq{ch}_{h}_{j}", tag="psa")
                        nc.tensor.matmul(
                            kq[:],
                            p["xr"][2 + h][:, cc * BLK + j * 128:
                                           cc * BLK + (j + 1) * 128],
                            qr, start=True, stop=True)
                        kqj = attp.tile([128, BLK], dt.float16,
                                        name=f"kqd{ch}_{h}_{j}", tag="kqd",
                                        bufs=4)
                        nc.vector.tensor_tensor(
                            out=kqj[:], in0=kq[:],
                            in1=diag[:, h, j, :], op=OP.mult)
                        kqd[h][j] = kqj
                    qt = attp.tile([128, BLK], dt.float16,
                                   name=f"qt{ch}_{h}", tag="qt")
                    nc.vector.tensor_tensor(out=qt[:], in0=qr,
                                            in1=qdec[:, h, :], op=OP.mult)
                    qts.append(qt)
                # phase 2: output + kv update per head
                for h in range(HPC):
                    kv_cur = kv_sb[h][ch % 2]
                    kv_nxt = kv_sb[h][(ch + 1) % 2]
                    kv16_cur = kv16[h][ch % 2]
                    kv16_nxt = kv16[h][(ch + 1) % 2]
                    ops = psa.tile([128, BLK], dt.float32,
                                   name=f"ops{ch}_{h}", tag="psa")
                    for j in range(2):
                        nc.tensor.matmul(
                            ops[:],
                            p["v_nat"][cc * 2 + j][:, h * 128:(h + 1) * 128],
                            kqd[h][j][:], start=(j == 0), stop=False)
                    nc.tensor.matmul(ops[:], kv16_cur[:], qts[h][:],
                                     start=False, stop=True)
                    # evict: square for group-norm + value copy
                    sqh = attp.tile([128, BLK], dt.float16,
                                    name=f"gsq{ch}_{h}", tag="gsq", bufs=2)
                    nc.scalar.activation(sqh[:], ops[:], AF.Square)
                    ob = attp.tile([128, BLK], dt.float16,
                                   name=f"ob{ch}_{h}", tag="ob", bufs=4)
                    nc.scalar.activation(ob[:], ops[:], AF.Identity)
                    obs[cc][h] = ob
                    nc.tensor.matmul(gssqs[cc][:], ones128[:], sqh[:],
                                     start=(h == 0), stop=(h == HPC - 1))
                    # kv update
                    kvp_ps = psa.tile([128, 128], dt.float32,
                                      name=f"kvp{ch}_{h}", tag="psa")
                    for j in range(2):
                        nc.tensor.matmul(
                            kvp_ps[:], knat[h][cc][j][:],
                            p["v_nat"][cc * 2 + j][:, h * 128:(h + 1) * 128],
                            start=(j == 0), stop=(j == 1))
                    nc.vector.scalar_tensor_tensor(
                        out=kv_nxt[:], in0=kv_cur[:], scalar=blkd[:, h:h + 1],
                        in1=kvp_ps[:], op0=OP.mult, op1=OP.add)
                    nc.vector.tensor_copy(kv16_nxt[:], kv_nxt[:])
            # batched group-norm for both chunks, then gate + send
            glts, grstds = [], []
            for cc in range(CPG):
                gb = attp.tile([1, BLK], dt.float32, name=f"gb{g}_{cc}",
                               tag="glt", bufs=2)
                nc.vector.tensor_scalar_add(gb[:], gssqs[cc][:],
                                            float(CPC * EPS))
                glt = attp.tile([1, BLK], dt.float32, name=f"glt{g}_{cc}",
                                tag="glt2", bufs=2)
                nc.vector.reciprocal_approx_fast(out=glt[:], in_=gb[:])
                glts.append(glt)
            for cc in range(CPG):
                grstd = attp.tile([1, BLK], dt.float16,
                                  name=f"grstd{g}_{cc}", tag="grstd", bufs=2)
                nc.scalar.activation(grstd[:], glts[cc][:], AF.Sqrt,
                                     scale=float(CPC))
                grstds.append(grstd)
            for cc in range(CPG):
                ch = g * CPG + cc
                gbc = attp.tile([128, BLK], dt.float16, name=f"gbc{ch}",
                                tag="gbc", bufs=2)
                nc.gpsimd.partition_broadcast(gbc[:], grstds[cc][:])
                for h in range(HPC):
                    y1 = yp.tile([128, BLK], dt.float16,
                                 name=f"y1{ch}_{h}", tag="y1")
                    nc.vector.scalar_tensor_tensor(
                        out=y1[:], in0=obs[cc][h][:], scalar=gnw[:, h:h + 1],
                        in1=gbc[:], op0=OP.mult, op1=OP.mult)
                    y2 = yp.tile([128, BLK], dt.float16,
                                 name=f"y2{ch}_{h}", tag="y2", bufs=2)
                    nc.vector.tensor_tensor(
                        out=y2[:], in0=y1[:],
                        in1=p["sg"][h][:, cc * BLK:(cc + 1) * BLK],
                        op=OP.mult)
                    for half in range(2):
                        u = 2 * ch + half
                        nc.sync.dma_start(
                            out=y_send[u // N_CORES][u % N_CORES,
                                                     h * 128:(h + 1) * 128,
                                                     :],
                            in_=y2[:, half * 128:(half + 1) * 128])
            if g % 2 == 1:
                pc = g // 2
                nc.gpsimd.collective_compute(
                    "AllToAll", mybir.AluOpType.bypass,
                    replica_groups=[list(range(N_CORES))],
                    ins=[y_send[pc][:].opt()],
                    outs=[y_recv[pc][:].opt()],
                )

        st_prev = None   # state of group b-1 awaiting norm/rope/attention
        st_next = None
        qkb = cload("qkb_s", qkb_d[:], [128, 4])
        # tiny collective up front: absorbs launch skew + first-op setup
        # during the startup DMA window instead of at the first real a2a
        nc.gpsimd.collective_compute(
            "AllToAll", mybir.AluOpType.bypass,
            replica_groups=[list(range(N_CORES))],
            ins=[dummy_s[:].opt()],
            outs=[dummy_r[:].opt()],
        )
        st0 = dma_group(0)
        # remaining wB column blocks in chain-consumption order
        for ci in range(1, 6):
            nc.sync.dma_start(out=wB_sb[:, :, ci * 128:(ci + 1) * 128],
                              in_=wB_r[:, :, ci * 128:(ci + 1) * 128])
        nc.sync.dma_start(out=wv_sb[:, 0:8, :], in_=wv_r[:, 0:8, :])
        nc.sync.dma_start(out=wv_sb[:, 8:KT, :], in_=wv_r[:, 8:KT, :])
        vb = cload("vb_s", vb_d[:], [128, 256], dt.float16)
        ones128 = cload("ones128_s", ones128_d[:], [128, 1], dt.float16)
        qdec = cload("qdec_s", qdec_d[:], [128, HPC, BLK], dt.float16)
        kdec = cload("kdec_s", kdec_d[:], [128, HPC, BLK], dt.float16)
        diag = cload("diag_s", diag_d[:], [128, HPC, 2, BLK], dt.float16)
        qnw = cload("qnw_s", qnw_d[:], [128, 1])
        knw = cload("knw_s", knw_d[:], [128, 1])
        gnw = cload("gnw_s", gnw_d[:], [128, HPC])
        blkd = cload("blkd_s", blk_d[:], [128, HPC])
        idm = cload("idm_s", idm_d[:], [128, 128], dt.float16)
        for h in range(HPC):
            nc.sync.dma_start(out=kv_sb[h][0][:], in_=zkv_d[:])
            nc.vector.memset(kv16[h][0][:], 0.0)

        for b in range(NG + 1):
            cur = None
            if b < NG:
                cur = st0 if b == 0 else st_next
                if b + 1 < NG:
                    st_next = dma_group(b + 1)
                hk = cur["hk"]
            if b == 0:
                # dense weights not needed until body 5; issue behind the
                # first two groups' inputs
                dw_r = dwT.rearrange("(t p) c -> p t c", p=128)
                for hh in range(2):
                    for q in range(2):
                        nc.sync.dma_start(
                            out=dwts[hh][:, q * 8:(q + 1) * 8, :],
                            in_=dw_r[:, q * 8:(q + 1) * 8,
                                     hh * 1024:(hh + 1) * 1024])

            p = st_prev  # group b-1 state

            if cur is None and p is not None:
                if not p.get("chain_done"):
                    emit_norm_a(p, 0)
                    emit_norm_a(p, 2)
                    emit_norm_c(p)
                    emit_swap(p, b)
                    emit_rope_tail(p)
                # piece 6's a2a landed two bodies ago: half its dense keeps
                # the PE busy ahead of the final attention; the rest is
                # emitted after the attention sends to cover the final
                # a2a's flight time
                dense_piece(6, (0, 1))

            if cur is not None:
                accs = []
                for ci in range(6):  # 0,1=q 2,3=k 4,5=g
                    acc = psp.tile([128, SEQ_G], dt.float32,
                                   name=f"acc{b}_{ci}", tag="ps")
                    proj_chain(None, ci, acc[:], hk)
                    accs.append(acc)
                    if ci == 0 and p is not None:
                        emit_norm_a(p, 0)
                    if ci == 1 and p is not None:
                        emit_norm_a(p, 2)
                    if ci == 2 and p is not None:
                        emit_norm_c(p)
                    if ci == 3 and p is not None:
                        emit_swap(p, b)
                        emit_rope_tail(p)
                    if ci < 4:
                        bcol = (ci % 2) * 2 + ci // 2
                        xb = evp.tile([128, SEQ_G], dt.float16,
                                      name=f"xb{b}_{ci}", tag="xb", bufs=7)
                        nc.scalar.activation(xb[:], acc[:], AF.Identity,
                                             bias=qkb[:, bcol:bcol + 1])
                        accs[ci] = xb
                    else:
                        th = xrp.tile([128, SEQ_G], dt.float16,
                                      name=f"th{b}_{ci}", tag=f"th{ci}",
                                      bufs=2)
                        nc.scalar.activation(th[:], acc[:], AF.Tanh,
                                             scale=0.5)
                        accs[ci] = th
                v_accs = []
                for s2 in range(2):
                    accv = psp.tile([128, SEQ_G], dt.float32,
                                    name=f"accv{b}_{s2}", tag="ps")
                    for half in range(2):
                        proj_chain(s2 * 2 + half, None,
                                   accv[:, half * 256:(half + 1) * 256], hk)
                    v_accs.append(accv)
                sq_t = []
                for ci in range(4):
                    sq = evp.tile([128, SEQ_G], dt.float16,
                                  name=f"sq{b}_{ci}", tag="sq", bufs=4)
                    nc.scalar.activation(sq[:], accs[ci][:], AF.Square)
                    sq_t.append(sq)

            if p is not None:
                emit_knat_attn(p)

            if cur is not None:
                v_nat = []
                for s2 in range(2):
                    for half in range(2):
                        st = s2 * 2 + half
                        vn = natp.tile([128, 256], dt.float16,
                                       name=f"vn{b}_{st}", tag=f"vn{st}",
                                       bufs=2)
                        nc.vector.tensor_tensor(
                            out=vn[:],
                            in0=v_accs[s2][:, half * 256:(half + 1) * 256],
                            in1=vb[:], op=OP.add)
                        v_nat.append(vn)

            if cur is not None:
                st_prev = {"g": b, "hk": hk, "cos": cur["cos"],
                           "sin": cur["sin"], "xb": accs[:4],
                           "th": accs[4:6], "v_nat": v_nat, "sq": sq_t}
                if b == NG - 1:
                    # pre-emit the last group's norm/rope chain so Act/DVE
                    # run it under this body's projections; body NG then
                    # starts its attention immediately
                    emit_norm_a(st_prev, 0)
                    emit_norm_a(st_prev, 2)
                    emit_norm_c(st_prev)
                    emit_swap(st_prev, b + 1)
                    emit_rope_tail(st_prev)
                    st_prev["chain_done"] = True

            if b >= 5 and b % 2 == 1:
                dense_piece((b - 5) // 2)

        dense_piece(6, (2, 3))
        dense_piece(NP - 1)
        ctx.close()

    nc.compile()
    return nc


def _make_emitters(nc, tc, pools, consts_d, mybir):
    pass  # placeholder (structure kept flat in _build_program)


def _stage(hidden_states, positions, qkv_w, qkv_b, q_norm_w, k_norm_w,
           g_w, g_norm_w, dense_w):
    f32 = np.float32
    f16 = np.float16
    hidden_states = np.asarray(hidden_states, dtype=f32)
    positions = np.asarray(positions)
    qkv_w = np.asarray(qkv_w, dtype=f32)
    qkv_b = np.asarray(qkv_b, dtype=f32)
    q_norm_w = np.asarray(q_norm_w, dtype=f32)
    k_norm_w = np.asarray(k_norm_w, dtype=f32)
    g_w = np.asarray(g_w, dtype=f32)
    g_norm_w = np.asarray(g_norm_w, dtype=f32)
    dense_w = np.asarray(dense_w, dtype=f32)
    slopes = _build_slopes()

    hsT = np.ascontiguousarray(hidden_states.T).astype(f16)

    inv_freq = 1.0 / (ROPE_THETA ** (np.arange(0, D, 2, dtype=f32) / D))
    freqs = positions.astype(f32)[:, None] * inv_freq[None, :]  # [S, 64]
    cos = np.cos(freqs).T     # [64, S]
    sin = np.sin(freqs).T
    cosf = np.ascontiguousarray(np.concatenate([cos, cos], axis=0)).astype(f16)
    sinf = np.ascontiguousarray(np.concatenate([-sin, sin], axis=0)).astype(f16)

    idx = np.arange(BLK, dtype=f32)
    dwT = np.ascontiguousarray(dense_w.T).astype(f16)
    ones128 = np.ones((128, 1), dtype=f16)
    idm = np.eye(128, dtype=f16)
    qnw = (q_norm_w * SCALE).reshape(128, 1).astype(f32)
    knw = k_norm_w.reshape(128, 1).copy()

    in_maps = []
    for j in range(N_CORES):
        heads = [j * HPC + h for h in range(HPC)]
        c0 = j * CPC
        wBm = np.zeros((KT * 128, 768), dtype=f16)
        wBm[:, 0:256] = qkv_w[c0:c0 + CPC, :].T
        wBm[:, 256:512] = qkv_w[HID + c0:HID + c0 + CPC, :].T
        wBm[:, 512:768] = g_w[c0:c0 + CPC, :].T
        wvm = np.ascontiguousarray(
            qkv_w[2 * HID + c0:2 * HID + c0 + CPC, :].T).astype(f16)
        # acc ci covers 128 channels: head ci%2 of q (ci<2) or k (ci in 2,3)
        qb = qkv_b[c0:c0 + CPC]
        kb = qkv_b[HID + c0:HID + c0 + CPC]
        qkbm = np.stack([qb[0:128], kb[0:128], qb[128:256], kb[128:256]],
                        axis=-1).astype(f32)
        vbias = qkv_b[2 * HID + c0:2 * HID + c0 + CPC]
        vb_bcast = np.ascontiguousarray(
            np.broadcast_to(vbias[None, :], (128, 256))).astype(f16)

        sl = slopes[heads]  # [HPC]
        qdec = np.exp(-sl[:, None] * (idx + 1.0)[None, :])
        qdec = np.ascontiguousarray(
            np.broadcast_to(qdec[None, :, :], (128, HPC, BLK))).astype(f16)
        kd = np.exp(-sl[:, None] * (BLK - 1.0 - idx)[None, :])
        kdecm = np.ascontiguousarray(
            np.broadcast_to(kd[None, :, :], (128, HPC, BLK))).astype(f16)
        dif = idx[:, None] - idx[None, :]           # [i, j]
        diagT = np.zeros((128, HPC, 2, BLK), dtype=f16)
        for hh in range(HPC):
            dd = np.where(
                dif >= 0,
                np.exp(-sl[hh] * np.where(dif >= 0, dif, 0.0)),
                0.0)                                # [i, j]
            ddT = dd.T.astype(f16)                  # [j, i]
            diagT[:, hh, 0, :] = ddT[0:128]
            diagT[:, hh, 1, :] = ddT[128:256]
        blkdec = np.ascontiguousarray(np.broadcast_to(
            np.exp(-sl * BLK).astype(f32)[None, :], (128, HPC)))
        gnwm = np.ascontiguousarray(g_norm_w[c0:c0 + CPC].reshape(HPC, 128).T)

        in_maps.append({
            "hsT": hsT, "wB": wBm, "wv": wvm, "dwT": dwT,
            "cosf": cosf, "sinf": sinf,
            "qdec": qdec, "kdec": kdecm, "diagT": diagT,
            "qnw": qnw, "knw": knw, "gnw": gnwm, "blkdec": blkdec,
            "qkb": qkbm, "vbias": vb_bcast,
            "ones128": ones128, "idm": idm,
            "zkv": np.zeros((128, 128), dtype=f32),
        })
    return in_maps


def _assemble(results):
    out = np.empty((S, HID), dtype=np.float32)
    for j in range(N_CORES):
        o = results[j]["out"]
        for p in range(NP):
            u = p * N_CORES + j
            out[u * 128:(u + 1) * 128] = o[p * 128:(p + 1) * 128]
    return out


def kernel(**inputs):
    from concourse.bass_utils import run_bass_kernel_spmd

    if "nc" not in _cache:
        _cache["nc"] = _build_program()
    nc = _cache["nc"]
    in_maps = _stage(**inputs)
    res = run_bass_kernel_spmd(nc, in_maps, list(range(N_CORES)))
    return _assemble(res.results)



# revision 47
# speedup vs baseline: 1.0067x; 1.0067x over previous
"""BailingMoE linear attention (lightning attention) on 8 trn2 NeuronCores.

Tensor-parallel over heads (2 heads/core), fp16 matmul operands, fp32
PSUM. Software-pipelined: body b runs the qkv/g projections of group b
on the PE while the RMSNorm/RoPE/attention chains of group b-1 run on
Act/DVE/GpSimd, so the PE never drains on the norm chains. All scalar
activations live in one HW table set (ln/exp/square/identity/copy):
the sigmoid gate is 1/(1+exp(-g)) with a DVE divide, so the activation
table is loaded exactly once. The RoPE half-swap runs as SBUF->SBUF
DMAs instead of PE permutation matmuls. Weight/input DMAs are split so
the first projection starts as soon as the first contraction tile
lands. The head->sequence AllToAll splits into 8 half-chunk pieces
(128 rows per core each) so collectives fire every other body and the
final piece's collective+dense tail is half as long.
"""
import math

import numpy as np

S = 8192
HID = 2048
H = 16
D = 128
BLK = 256
GROUPS = 8
EPS = 1e-5
ROPE_THETA = 600000.0
SCALE = D ** -0.5
N_CORES = 8
HPC = H // N_CORES          # heads per core = 2
CPC = HPC * D               # channels per core = 256
KT = 16                     # contraction tiles (2048 hid; bias via ACT)
SEQ_G = 512                 # seq per projection group
NG = S // SEQ_G             # 16 groups
SB = S // N_CORES           # seq rows per core in the output = 1024
CPG = SEQ_G // BLK          # chunks per group = 2
NCH = S // BLK              # 32 chunks
NP = 8                      # a2a/dense pieces (8 x 128-row units/core)

_cache = {}


def _build_slopes():
    start = 2.0 ** (-(2.0 ** (-(math.log2(H) - 3.0))))
    slopes = np.array([start * start ** i for i in range(H)], dtype=np.float32)
    return slopes * np.float32(1.0 - 0.0 / (20 - 1) + 1e-5)


def _build_program():
    import concourse.bacc as bacc
    import concourse.tile as tile
    import concourse.mybir as mybir
    from contextlib import ExitStack

    dt = mybir.dt
    AF = mybir.ActivationFunctionType
    OP = mybir.AluOpType

    nc = bacc.Bacc("TRN2", target_bir_lowering=False, debug=False,
                   num_devices=N_CORES)

    def din(name, shape, dtype=dt.float32):
        return nc.dram_tensor(name, shape, dtype, kind="ExternalInput").ap()

    hsT = din("hsT", [KT * 128, S], dt.float16)
    wB = din("wB", [KT * 128, 768], dt.float16)   # cols: q(256) k(256) g(256)
    wv = din("wv", [KT * 128, 256], dt.float16)
    dwT = din("dwT", [HID, HID], dt.float16)
    cosf = din("cosf", [128, S], dt.float16)       # [cos; cos]
    sinf = din("sinf", [128, S], dt.float16)       # [-sin; sin]
    qdec_d = din("qdec", [128, HPC, BLK], dt.float16)
    kdec_d = din("kdec", [128, HPC, BLK], dt.float16)
    diag_d = din("diagT", [128, HPC, 2, BLK], dt.float16)
    qnw_d = din("qnw", [128, 1])                   # q_norm_w * SCALE
    knw_d = din("knw", [128, 1])
    qkb_d = din("qkb", [128, 4])                   # qb0 kb0 qb1 kb1
    vb_d = din("vbias", [128, 256], dt.float16)    # v bias bcast over seq
    gnw_d = din("gnw", [128, HPC])
    blk_d = din("blkdec", [128, HPC])
    ones128_d = din("ones128", [128, 1], dt.float16)
    idm_d = din("idm", [128, 128], dt.float16)
    zkv_d = din("zkv", [128, 128], dt.float32)

    out_d = nc.dram_tensor("out", [SB, HID], dt.float16,
                           kind="ExternalOutput").ap()

    with tile.TileContext(nc) as tc:
        ctx = ExitStack()
        consts = ctx.enter_context(tc.tile_pool(name="consts", bufs=1))
        wpool = ctx.enter_context(tc.tile_pool(name="wpool", bufs=1))
        dramp = ctx.enter_context(tc.tile_pool(name="dramp", bufs=1,
                                               space="DRAM"))
        y_send = [dramp.tile([N_CORES, CPC, 128], dt.float16,
                             name=f"y_send{i}", tag=f"y_send{i}")
                  for i in range(NP)]
        y_recv = [dramp.tile([N_CORES, CPC, 128], dt.float16,
                             name=f"y_recv{i}", tag=f"y_recv{i}")
                  for i in range(NP)]

        def cload(name, ap_src, shape, dtype=dt.float32):
            t = consts.tile(shape, dtype, name=name, tag=name)
            nc.sync.dma_start(out=t[:], in_=ap_src)
            return t

        # weights first, split by output-column block so chain ci starts
        # as soon as its 0.5MB slice lands; everything not needed by the
        # first projection group is issued after dma_group(0)
        wB_sb = wpool.tile([128, KT, 768], dt.float16, name="wB_sb")
        wB_r = wB.rearrange("(t p) c -> p t c", p=128)
        nc.sync.dma_start(out=wB_sb[:, :, 0:128], in_=wB_r[:, :, 0:128])
        wv_sb = wpool.tile([128, KT, 256], dt.float16, name="wv_sb")
        wv_r = wv.rearrange("(t p) c -> p t c", p=128)
        dwts = [wpool.tile([128, KT, 1024], dt.float16, name=f"dwt{hh}",
                           tag=f"dw{hh}") for hh in range(2)]
        dummy_s = dramp.tile([N_CORES, 4], dt.float16, name="dummy_s",
                             tag="dummy_s")
        dummy_r = dramp.tile([N_CORES, 4], dt.float16, name="dummy_r",
                             tag="dummy_r")


        hkp = ctx.enter_context(tc.tile_pool(name="hkp", bufs=2))
        tabp = ctx.enter_context(tc.tile_pool(name="tabp", bufs=1))
        evp = ctx.enter_context(tc.tile_pool(name="evp", bufs=2))
        xrp = ctx.enter_context(tc.tile_pool(name="xrp", bufs=1))
        natp = ctx.enter_context(tc.tile_pool(name="natp", bufs=1))
        attp = ctx.enter_context(tc.tile_pool(name="attp", bufs=2))
        kvpl = ctx.enter_context(tc.tile_pool(name="kvpl", bufs=1))
        yp = ctx.enter_context(tc.tile_pool(name="yp", bufs=2))
        dyp = ctx.enter_context(tc.tile_pool(name="dyp", bufs=1))
        dop = ctx.enter_context(tc.tile_pool(name="dop", bufs=1))
        psp = ctx.enter_context(tc.tile_pool(name="psp", bufs=2, space="PSUM"))
        pse = ctx.enter_context(tc.tile_pool(name="pse", bufs=2, space="PSUM"))
        psa = ctx.enter_context(tc.tile_pool(name="psa", bufs=2, space="PSUM"))
        dps = ctx.enter_context(tc.tile_pool(name="dps", bufs=2, space="PSUM"))

        # persistent kv state: fp32 master + fp16 matmul copy, ping-pong
        kv_sb = [[kvpl.tile([128, 128], dt.float32, name=f"kv{h}_{i}",
                            tag=f"kv{h}_{i}") for i in range(2)]
                 for h in range(HPC)]
        kv16 = [[kvpl.tile([128, 128], dt.float16, name=f"kv16_{h}_{i}",
                           tag=f"kv16_{h}_{i}") for i in range(2)]
                for h in range(HPC)]

        yt_cache = {}

        def dense_piece(p, hqs=(0, 1, 2, 3)):
            if p in yt_cache:
                yt = yt_cache[p]
            else:
                yt = dyp.tile([128, N_CORES, 2, 128], dt.float16,
                              name=f"yt{p}", tag="yt")
                nc.sync.dma_start(
                    out=yt[:],
                    in_=y_recv[p].rearrange("s (h p) q -> p s h q", p=128))
                yt_cache[p] = yt
            for hq in hqs:
                acc = dps.tile([128, 512], dt.float32,
                               name=f"dacc{p}_{hq}", tag="dps")
                for ct in range(2 * N_CORES):
                    nc.tensor.matmul(
                        acc[:],
                        yt[:, ct // 2, ct % 2, :],
                        dwts[hq // 2][:, ct,
                                      (hq % 2) * 512:(hq % 2 + 1) * 512],
                        start=(ct == 0), stop=(ct == 2 * N_CORES - 1))
                ot = dop.tile([128, 512], dt.float16,
                              name=f"ot{p}_{hq}", tag="ot", bufs=2)
                nc.scalar.activation(ot[:], acc[:], AF.Copy)
                srow = p * 128
                nc.sync.dma_start(
                    out=out_d[srow:srow + 128,
                              hq * 512:(hq + 1) * 512],
                    in_=ot[:])

        def dma_group(g):
            hk = hkp.tile([128, KT, SEQ_G], dt.float16,
                          name=f"hk{g}", tag="hk")
            hs_r = hsT.rearrange("(t p) s -> p t s", p=128)
            for i in range(4):
                nc.sync.dma_start(
                    out=hk[:, i * 4:(i + 1) * 4, :],
                    in_=hs_r[:, i * 4:(i + 1) * 4,
                             g * SEQ_G:(g + 1) * SEQ_G])
            cos_g = tabp.tile([128, SEQ_G], dt.float16, name=f"cos{g}",
                              tag="cos")
            nc.sync.dma_start(out=cos_g[:],
                              in_=cosf[:, g * SEQ_G:(g + 1) * SEQ_G])
            sin_g = tabp.tile([128, SEQ_G], dt.float16, name=f"sin{g}",
                              tag="sin")
            nc.sync.dma_start(out=sin_g[:],
                              in_=sinf[:, g * SEQ_G:(g + 1) * SEQ_G])
            return {"hk": hk, "cos": cos_g, "sin": sin_g}

        def proj_chain(st, ci, acc_slice, hk):
            # one 16-matmul accumulation writing acc_slice
            if ci is not None:   # q/k/g chain: weights stationary
                for t in range(KT):
                    nc.tensor.matmul(
                        acc_slice, wB_sb[:, t, ci * 128:(ci + 1) * 128],
                        hk[:, t, :], start=(t == 0), stop=(t == KT - 1))
            else:                # v chain: hk seq-slice stationary
                for t in range(KT):
                    nc.tensor.matmul(
                        acc_slice, hk[:, t, st * 128:(st + 1) * 128],
                        wv_sb[:, t, :], start=(t == 0), stop=(t == KT - 1))

        epsb4 = consts.tile([4, 1], dt.float32, name="epsb4", tag="epsb4")
        nc.vector.memset(epsb4[:], EPS)

        def emit_norm_a(p, cj0):
            # two sum-of-squares matmuls + fast DVE reciprocal (fp32)
            for cj in (cj0, cj0 + 1):
                ssq = psa.tile([1, SEQ_G], dt.float32,
                               name=f"ssq{p['g']}_{cj}", tag="psa")
                lnt = evp.tile([1, SEQ_G], dt.float32,
                               name=f"ln{p['g']}_{cj}", tag="ln", bufs=3)
                nc.tensor.matmul(ssq[:], ones128[:], p["sq"][cj][:],
                                 start=True, stop=True)
                nc.vector.reciprocal_approx_fast(out=lnt[:], in_=ssq[:])
                p.setdefault("ln", []).append(lnt)

        def emit_norm_c(p):
            rstds = []
            for cj in range(4):
                rstd = evp.tile([1, SEQ_G], dt.float16,
                                name=f"rstd{p['g']}_{cj}", tag="rstd", bufs=2)
                # rstd = sqrt(D/ssq); eps negligible vs ssq/D ~ O(1)
                nc.scalar.activation(rstd[:], p["ln"][cj][:], AF.Sqrt,
                                     scale=float(D))
                rstds.append(rstd)
            p["sg"] = []
            for i in range(2):
                sg = xrp.tile([128, SEQ_G], dt.float16,
                              name=f"sg_{p['g']}_{i}", tag=f"th1{i}",
                              bufs=1)
                nc.vector.tensor_scalar(sg[:], p["th"][i][:], 0.5, 0.5,
                                        OP.mult, OP.add)
                p["sg"].append(sg)
            p["xn"] = []
            p["m1"] = []
            for cj in range(4):
                rbc = evp.tile([128, SEQ_G], dt.float16,
                               name=f"rbc{p['g']}_{cj}", tag="rbc", bufs=2)
                nc.gpsimd.partition_broadcast(rbc[:], rstds[cj][:])
                xn = evp.tile([128, SEQ_G], dt.float16,
                              name=f"xn{p['g']}_{cj}", tag="xn", bufs=4)
                nc.vector.scalar_tensor_tensor(
                    out=xn[:], in0=p["xb"][cj][:],
                    scalar=qnw[:] if cj < 2 else knw[:],
                    in1=rbc[:], op0=OP.mult, op1=OP.mult)
                p["xn"].append(xn)
                m1 = evp.tile([128, SEQ_G], dt.float16,
                              name=f"m1{p['g']}_{cj}", tag="m1", bufs=3)
                nc.vector.tensor_tensor(out=m1[:], in0=xn[:],
                                        in1=p["cos"][:], op=OP.mult)
                p["m1"].append(m1)

        def emit_swap(p, b):
            # RoPE half-swap via SBUF->SBUF DMA (sign baked into sinf)
            p["swp"] = []
            for cj in range(4):
                swp = xrp.tile([128, SEQ_G], dt.float16,
                               name=f"swp{b}_{cj}", tag=f"swp{cj % 2}",
                               bufs=1)
                nc.sync.dma_start(out=swp[0:64, :], in_=p["xn"][cj][64:128, :])
                nc.sync.dma_start(out=swp[64:128, :], in_=p["xn"][cj][0:64, :])
                p["swp"].append(swp)

        def emit_rope_tail(p):
            p["xr"] = []   # cj 0,1 = q heads; 2,3 = k heads
            for cj in range(4):
                m2 = evp.tile([128, SEQ_G], dt.float16,
                              name=f"m2{p['g']}_{cj}", tag="m2", bufs=1)
                nc.vector.tensor_tensor(out=m2[:], in0=p["swp"][cj][:],
                                        in1=p["sin"][:], op=OP.mult)
                xr = xrp.tile([128, SEQ_G], dt.float16,
                              name=f"xr{p['g']}_{cj}", tag=f"xr{cj}", bufs=1)
                nc.vector.tensor_tensor(out=xr[:], in0=p["m1"][cj][:],
                                        in1=m2[:], op=OP.add)
                p["xr"].append(xr)
            p["ktil"] = []
            for h in range(HPC):
                ktil = xrp.tile([128, SEQ_G], dt.float16,
                                name=f"ktil{p['g']}_{h}", tag=f"ktil{h}",
                                bufs=1)
                for cc in range(CPG):
                    nc.vector.tensor_tensor(
                        out=ktil[:, cc * BLK:(cc + 1) * BLK],
                        in0=p["xr"][2 + h][:, cc * BLK:(cc + 1) * BLK],
                        in1=kdec[:, h, :], op=OP.mult)
                p["ktil"].append(ktil)

        def emit_knat_attn(p):
            g = p["g"]
            knat = [[None] * 2 for _ in range(HPC)]
            for h in range(HPC):
                for cc in range(CPG):
                    for j in range(2):
                        tp = pse.tile([128, 128], dt.float16,
                                      name=f"tp{g}_{h}_{cc}_{j}", tag="swp",
                                      bufs=2)
                        nc.tensor.transpose(
                            tp[:],
                            p["ktil"][h][:, cc * BLK + j * 128:
                                         cc * BLK + (j + 1) * 128],
                            idm[:])
                        kn = natp.tile([128, 128], dt.float16,
                                       name=f"kn{g}_{h}_{cc}_{j}", tag="kn",
                                       bufs=6)
                        nc.vector.tensor_copy(kn[:], tp[:])
                        if knat[h][cc] is None:
                            knat[h][cc] = []
                        knat[h][cc].append(kn)
            gssqs = [pse.tile([1, BLK], dt.float32, name=f"gssq{g}_{cc}",
                              tag="swp") for cc in range(CPG)]
            obs = [[None] * HPC for _ in range(CPG)]
            for cc in range(CPG):
                ch = g * CPG + cc
                # phase 1: kq + masks + decayed q, both heads
                kqd = [[None, None] for _ in range(HPC)]
                qts = []
                for h in range(HPC):
                    qr = p["xr"][h][:, cc * BLK:(cc + 1) * BLK]
                    for j in range(2):
                        kq = psa.tile([128, BLK], dt.float32,
                                      name=f"kq{ch}_{h}_{j}", tag="psa")
                        nc.tensor.matmul(
                            kq[:],
                            p["xr"][2 + h][:, cc * BLK + j * 128:
                                           cc * BLK + (j + 1) * 128],
                            qr, start=True, stop=True)
                        kqj = attp.tile([128, BLK], dt.float16,
                                        name=f"kqd{ch}_{h}_{j}", tag="kqd",
                                        bufs=4)
                        nc.vector.tensor_tensor(
                            out=kqj[:], in0=kq[:],
                            in1=diag[:, h, j, :], op=OP.mult)
                        kqd[h][j] = kqj
                    qt = attp.tile([128, BLK], dt.float16,
                                   name=f"qt{ch}_{h}", tag="qt")
                    nc.vector.tensor_tensor(out=qt[:], in0=qr,
                                            in1=qdec[:, h, :], op=OP.mult)
                    qts.append(qt)
                # phase 2: output + kv update per head
                for h in range(HPC):
                    kv_cur = kv_sb[h][ch % 2]
                    kv_nxt = kv_sb[h][(ch + 1) % 2]
                    kv16_cur = kv16[h][ch % 2]
                    kv16_nxt = kv16[h][(ch + 1) % 2]
                    ops = psa.tile([128, BLK], dt.float32,
                                   name=f"ops{ch}_{h}", tag="psa")
                    for j in range(2):
                        nc.tensor.matmul(
                            ops[:],
                            p["v_nat"][cc * 2 + j][:, h * 128:(h + 1) * 128],
                            kqd[h][j][:], start=(j == 0), stop=False)
                    nc.tensor.matmul(ops[:], kv16_cur[:], qts[h][:],
                                     start=False, stop=True)
                    # evict: square for group-norm + value copy
                    sqh = attp.tile([128, BLK], dt.float16,
                                    name=f"gsq{ch}_{h}", tag="gsq", bufs=2)
                    nc.scalar.activation(sqh[:], ops[:], AF.Square)
                    ob = attp.tile([128, BLK], dt.float16,
                                   name=f"ob{ch}_{h}", tag="ob", bufs=4)
                    nc.scalar.activation(ob[:], ops[:], AF.Identity)
                    obs[cc][h] = ob
                    nc.tensor.matmul(gssqs[cc][:], ones128[:], sqh[:],
                                     start=(h == 0), stop=(h == HPC - 1))
                    # kv update
                    kvp_ps = psa.tile([128, 128], dt.float32,
                                      name=f"kvp{ch}_{h}", tag="psa")
                    for j in range(2):
                        nc.tensor.matmul(
                            kvp_ps[:], knat[h][cc][j][:],
                            p["v_nat"][cc * 2 + j][:, h * 128:(h + 1) * 128],
                            start=(j == 0), stop=(j == 1))
                    nc.vector.scalar_tensor_tensor(
                        out=kv_nxt[:], in0=kv_cur[:], scalar=blkd[:, h:h + 1],
                        in1=kvp_ps[:], op0=OP.mult, op1=OP.add)
                    nc.vector.tensor_copy(kv16_nxt[:], kv_nxt[:])
            # batched group-norm for both chunks, then gate + send
            glts, grstds = [], []
            for cc in range(CPG):
                gb = attp.tile([1, BLK], dt.float32, name=f"gb{g}_{cc}",
                               tag="glt", bufs=2)
                nc.vector.tensor_scalar_add(gb[:], gssqs[cc][:],
                                            float(CPC * EPS))
                glt = attp.tile([1, BLK], dt.float32, name=f"glt{g}_{cc}",
                                tag="glt2", bufs=2)
                nc.vector.reciprocal_approx_fast(out=glt[:], in_=gb[:])
                glts.append(glt)
            for cc in range(CPG):
                grstd = attp.tile([1, BLK], dt.float16,
                                  name=f"grstd{g}_{cc}", tag="grstd", bufs=2)
                nc.scalar.activation(grstd[:], glts[cc][:], AF.Sqrt,
                                     scale=float(CPC))
                grstds.append(grstd)
            for cc in range(CPG):
                ch = g * CPG + cc
                gbc = attp.tile([128, BLK], dt.float16, name=f"gbc{ch}",
                                tag="gbc", bufs=2)
                nc.gpsimd.partition_broadcast(gbc[:], grstds[cc][:])
                for h in range(HPC):
                    y1 = yp.tile([128, BLK], dt.float16,
                                 name=f"y1{ch}_{h}", tag="y1")
                    nc.vector.scalar_tensor_tensor(
                        out=y1[:], in0=obs[cc][h][:], scalar=gnw[:, h:h + 1],
                        in1=gbc[:], op0=OP.mult, op1=OP.mult)
                    y2 = yp.tile([128, BLK], dt.float16,
                                 name=f"y2{ch}_{h}", tag="y2", bufs=2)
                    nc.vector.tensor_tensor(
                        out=y2[:], in0=y1[:],
                        in1=p["sg"][h][:, cc * BLK:(cc + 1) * BLK],
                        op=OP.mult)
                    for half in range(2):
                        u = 2 * ch + half
                        nc.sync.dma_start(
                            out=y_send[u // N_CORES][u % N_CORES,
                                                     h * 128:(h + 1) * 128,
                                                     :],
                            in_=y2[:, half * 128:(half + 1) * 128])
            if g % 2 == 1:
                pc = g // 2
                nc.gpsimd.collective_compute(
                    "AllToAll", mybir.AluOpType.bypass,
                    replica_groups=[list(range(N_CORES))],
                    ins=[y_send[pc][:].opt()],
                    outs=[y_recv[pc][:].opt()],
                )

        st_prev = None   # state of group b-1 awaiting norm/rope/attention
        st_next = None
        qkb = cload("qkb_s", qkb_d[:], [128, 4])
        # tiny collective up front: absorbs launch skew + first-op setup
        # during the startup DMA window instead of at the first real a2a
        nc.gpsimd.collective_compute(
            "AllToAll", mybir.AluOpType.bypass,
            replica_groups=[list(range(N_CORES))],
            ins=[dummy_s[:].opt()],
            outs=[dummy_r[:].opt()],
        )
        st0 = dma_group(0)
        # remaining wB column blocks in chain-consumption order
        for ci in range(1, 6):
            nc.sync.dma_start(out=wB_sb[:, :, ci * 128:(ci + 1) * 128],
                              in_=wB_r[:, :, ci * 128:(ci + 1) * 128])
        nc.sync.dma_start(out=wv_sb[:, 0:8, :], in_=wv_r[:, 0:8, :])
        nc.sync.dma_start(out=wv_sb[:, 8:KT, :], in_=wv_r[:, 8:KT, :])
        vb = cload("vb_s", vb_d[:], [128, 256], dt.float16)
        ones128 = cload("ones128_s", ones128_d[:], [128, 1], dt.float16)
        qdec = cload("qdec_s", qdec_d[:], [128, HPC, BLK], dt.float16)
        kdec = cload("kdec_s", kdec_d[:], [128, HPC, BLK], dt.float16)
        diag = cload("diag_s", diag_d[:], [128, HPC, 2, BLK], dt.float16)
        qnw = cload("qnw_s", qnw_d[:], [128, 1])
        knw = cload("knw_s", knw_d[:], [128, 1])
        gnw = cload("gnw_s", gnw_d[:], [128, HPC])
        blkd = cload("blkd_s", blk_d[:], [128, HPC])
        idm = cload("idm_s", idm_d[:], [128, 128], dt.float16)
        for h in range(HPC):
            nc.sync.dma_start(out=kv_sb[h][0][:], in_=zkv_d[:])
            nc.vector.memset(kv16[h][0][:], 0.0)

        for b in range(NG + 1):
            cur = None
            if b < NG:
                cur = st0 if b == 0 else st_next
                if b + 1 < NG:
                    st_next = dma_group(b + 1)
                hk = cur["hk"]
            if b == 0:
                # dense weights not needed until body 5; issue behind the
                # first two groups' inputs
                dw_r = dwT.rearrange("(t p) c -> p t c", p=128)
                for hh in range(2):
                    for q in range(2):
                        nc.sync.dma_start(
                            out=dwts[hh][:, q * 8:(q + 1) * 8, :],
                            in_=dw_r[:, q * 8:(q + 1) * 8,
                                     hh * 1024:(hh + 1) * 1024])

            p = st_prev  # group b-1 state

            if cur is None and p is not None:
                if not p.get("chain_done"):
                    emit_norm_a(p, 0)
                    emit_norm_a(p, 2)
                    emit_norm_c(p)
                    emit_swap(p, b)
                    emit_rope_tail(p)


            if cur is not None:
                accs = []
                for ci in range(6):  # 0,1=q 2,3=k 4,5=g
                    acc = psp.tile([128, SEQ_G], dt.float32,
                                   name=f"acc{b}_{ci}", tag="ps")
                    proj_chain(None, ci, acc[:], hk)
                    accs.append(acc)
                    if ci == 0 and p is not None:
                        emit_norm_a(p, 0)
                    if ci == 1 and p is not None:
                        emit_norm_a(p, 2)
                    if ci == 2 and p is not None:
                        emit_norm_c(p)
                    if ci == 3 and p is not None:
                        emit_swap(p, b)
                        emit_rope_tail(p)
                    if ci < 4:
                        bcol = (ci % 2) * 2 + ci // 2
                        xb = evp.tile([128, SEQ_G], dt.float16,
                                      name=f"xb{b}_{ci}", tag="xb", bufs=7)
                        nc.scalar.activation(xb[:], acc[:], AF.Identity,
                                             bias=qkb[:, bcol:bcol + 1])
                        accs[ci] = xb
                    else:
                        th = xrp.tile([128, SEQ_G], dt.float16,
                                      name=f"th{b}_{ci}", tag=f"th{ci}",
                                      bufs=2)
                        nc.scalar.activation(th[:], acc[:], AF.Tanh,
                                             scale=0.5)
                        accs[ci] = th
                v_accs = []
                for s2 in range(2):
                    accv = psp.tile([128, SEQ_G], dt.float32,
                                    name=f"accv{b}_{s2}", tag="ps")
                    for half in range(2):
                        proj_chain(s2 * 2 + half, None,
                                   accv[:, half * 256:(half + 1) * 256], hk)
                    v_accs.append(accv)
                sq_t = []
                for ci in range(4):
                    sq = evp.tile([128, SEQ_G], dt.float16,
                                  name=f"sq{b}_{ci}", tag="sq", bufs=4)
                    nc.scalar.activation(sq[:], accs[ci][:], AF.Square)
                    sq_t.append(sq)

            if p is not None:
                emit_knat_attn(p)

            if cur is not None:
                v_nat = []
                for s2 in range(2):
                    for half in range(2):
                        st = s2 * 2 + half
                        vn = natp.tile([128, 256], dt.float16,
                                       name=f"vn{b}_{st}", tag=f"vn{st}",
                                       bufs=2)
                        nc.vector.tensor_tensor(
                            out=vn[:],
                            in0=v_accs[s2][:, half * 256:(half + 1) * 256],
                            in1=vb[:], op=OP.add)
                        v_nat.append(vn)

            if cur is not None:
                st_prev = {"g": b, "hk": hk, "cos": cur["cos"],
                           "sin": cur["sin"], "xb": accs[:4],
                           "th": accs[4:6], "v_nat": v_nat, "sq": sq_t}
                if b == NG - 1:
                    # pre-emit the last group's norm/rope chain so Act/DVE
                    # run it under this body's projections; body NG then
                    # starts its attention immediately
                    emit_norm_a(st_prev, 0)
                    emit_norm_a(st_prev, 2)
                    emit_norm_c(st_prev)
                    emit_swap(st_prev, b + 1)
                    emit_rope_tail(st_prev)
                    st_prev["chain_done"] = True

            if b >= 5 and b % 2 == 1 and b <= 13:
                dense_piece((b - 5) // 2)

        # pieces 5-7 deferred: their matmuls queue behind the final
        # attention's sends, covering the last a2a's flight time
        dense_piece(5)
        dense_piece(6)
        dense_piece(NP - 1)
        ctx.close()

    nc.compile()
    return nc


def _make_emitters(nc, tc, pools, consts_d, mybir):
    pass  # placeholder (structure kept flat in _build_program)


def _stage(hidden_states, positions, qkv_w, qkv_b, q_norm_w, k_norm_w,
           g_w, g_norm_w, dense_w):
    f32 = np.float32
    f16 = np.float16
    hidden_states = np.asarray(hidden_states, dtype=f32)
    positions = np.asarray(positions)
    qkv_w = np.asarray(qkv_w, dtype=f32)
    qkv_b = np.asarray(qkv_b, dtype=f32)
    q_norm_w = np.asarray(q_norm_w, dtype=f32)
    k_norm_w = np.asarray(k_norm_w, dtype=f32)
    g_w = np.asarray(g_w, dtype=f32)
    g_norm_w = np.asarray(g_norm_w, dtype=f32)
    dense_w = np.asarray(dense_w, dtype=f32)
    slopes = _build_slopes()

    hsT = np.ascontiguousarray(hidden_states.T).astype(f16)

    inv_freq = 1.0 / (ROPE_THETA ** (np.arange(0, D, 2, dtype=f32) / D))
    freqs = positions.astype(f32)[:, None] * inv_freq[None, :]  # [S, 64]
    cos = np.cos(freqs).T     # [64, S]
    sin = np.sin(freqs).T
    cosf = np.ascontiguousarray(np.concatenate([cos, cos], axis=0)).astype(f16)
    sinf = np.ascontiguousarray(np.concatenate([-sin, sin], axis=0)).astype(f16)

    idx = np.arange(BLK, dtype=f32)
    dwT = np.ascontiguousarray(dense_w.T).astype(f16)
    ones128 = np.ones((128, 1), dtype=f16)
    idm = np.eye(128, dtype=f16)
    qnw = (q_norm_w * SCALE).reshape(128, 1).astype(f32)
    knw = k_norm_w.reshape(128, 1).copy()

    in_maps = []
    for j in range(N_CORES):
        heads = [j * HPC + h for h in range(HPC)]
        c0 = j * CPC
        wBm = np.zeros((KT * 128, 768), dtype=f16)
        wBm[:, 0:256] = qkv_w[c0:c0 + CPC, :].T
        wBm[:, 256:512] = qkv_w[HID + c0:HID + c0 + CPC, :].T
        wBm[:, 512:768] = g_w[c0:c0 + CPC, :].T
        wvm = np.ascontiguousarray(
            qkv_w[2 * HID + c0:2 * HID + c0 + CPC, :].T).astype(f16)
        # acc ci covers 128 channels: head ci%2 of q (ci<2) or k (ci in 2,3)
        qb = qkv_b[c0:c0 + CPC]
        kb = qkv_b[HID + c0:HID + c0 + CPC]
        qkbm = np.stack([qb[0:128], kb[0:128], qb[128:256], kb[128:256]],
                        axis=-1).astype(f32)
        vbias = qkv_b[2 * HID + c0:2 * HID + c0 + CPC]
        vb_bcast = np.ascontiguousarray(
            np.broadcast_to(vbias[None, :], (128, 256))).astype(f16)

        sl = slopes[heads]  # [HPC]
        qdec = np.exp(-sl[:, None] * (idx + 1.0)[None, :])
        qdec = np.ascontiguousarray(
            np.broadcast_to(qdec[None, :, :], (128, HPC, BLK))).astype(f16)
        kd = np.exp(-sl[:, None] * (BLK - 1.0 - idx)[None, :])
        kdecm = np.ascontiguousarray(
            np.broadcast_to(kd[None, :, :], (128, HPC, BLK))).astype(f16)
        dif = idx[:, None] - idx[None, :]           # [i, j]
        diagT = np.zeros((128, HPC, 2, BLK), dtype=f16)
        for hh in range(HPC):
            dd = np.where(
                dif >= 0,
                np.exp(-sl[hh] * np.where(dif >= 0, dif, 0.0)),
                0.0)                                # [i, j]
            ddT = dd.T.astype(f16)                  # [j, i]
            diagT[:, hh, 0, :] = ddT[0:128]
            diagT[:, hh, 1, :] = ddT[128:256]
        blkdec = np.ascontiguousarray(np.broadcast_to(
            np.exp(-sl * BLK).astype(f32)[None, :], (128, HPC)))
        gnwm = np.ascontiguousarray(g_norm_w[c0:c0 + CPC].reshape(HPC, 128).T)

        in_maps.append({
            "hsT": hsT, "wB": wBm, "wv": wvm, "dwT": dwT,
            "cosf": cosf, "sinf": sinf,
            "qdec": qdec, "kdec": kdecm, "diagT": diagT,
            "qnw": qnw, "knw": knw, "gnw": gnwm, "blkdec": blkdec,
            "qkb": qkbm, "vbias": vb_bcast,
            "ones128": ones128, "idm": idm,
            "zkv": np.zeros((128, 128), dtype=f32),
        })
    return in_maps


def _assemble(results):
    out = np.empty((S, HID), dtype=np.float32)
    for j in range(N_CORES):
        o = results[j]["out"]
        for p in range(NP):
            u = p * N_CORES + j
            out[u * 128:(u + 1) * 128] = o[p * 128:(p + 1) * 128]
    return out


def kernel(**inputs):
    from concourse.bass_utils import run_bass_kernel_spmd

    if "nc" not in _cache:
        _cache["nc"] = _build_program()
    nc = _cache["nc"]
    in_maps = _stage(**inputs)
    res = run_bass_kernel_spmd(nc, in_maps, list(range(N_CORES)))
    return _assemble(res.results)

